# revision 17
# baseline (speedup 1.0000x reference)
"""Trainium2 Bass kernel for nn_CrossPath (sparse_attention).

Strategy (hardcoded for B=32, N=1024, C=256, H=8, d=32, CLS=6):
  - Data-parallel over batch: 8 NeuronCores x 4 batches each, params replicated.
  - All heavy matmuls run on the PE at 1 cycle/row: bf16 for proj/attention
    paths, float32r (tf32-like) for the precision-sensitive end_proj1.
  - The linear-attention context is computed via the Gram-matrix identity
        ctx = Wk^T (act^T act) Wv * scale
    which cuts both PE work and PSUM-evacuation volume vs materializing k/v.
  - Per-head softmax over d is done on [128,128] feature blocks with an
    additive block-diagonal mask; off-diagonal (cross-head) entries go to
    exp(-100)=0, so the transposed context is exactly block-diagonal and
    attends can use full 128-wide contractions.
  - Batches are software-pipelined: batch b's attention/output tail is
    interleaved into batch b+1's projection phase so the in-order PE never
    waits on vector-engine work.
  - Host-side glue: shard/transpose/cast/pack inputs, fold constant scales
    ((z1+v1)/2 -> 0.5 into ctx1 norm and Wp1's u-half), unshard outputs.
"""

import sys

sys.path.insert(0, "/opt/trn_rl_repo")

import numpy as np
import ml_dtypes
from contextlib import ExitStack

import concourse.bass as bass
import concourse.bacc as bacc
import concourse.tile as tile
import concourse.mybir as mybir
from concourse.bass_utils import run_bass_kernel_spmd

F32 = mybir.dt.float32
F32R = mybir.dt.float32r
BF16 = mybir.dt.bfloat16
AF = mybir.ActivationFunctionType
OP = mybir.AluOpType
BF = ml_dtypes.bfloat16

N_CORES = 8
B = 32
BPC = B // N_CORES  # batches per core
DIM = 256
N = 1024
H = 8
CLS = 6
SCALE = float((DIM // H) ** -0.5)
EPS = 1e-5
NT = N // 128  # 8 token tiles
MASKV = -100.0

_CACHE = {}


def _patch_act_tables():
    """Force all ACT functions onto the natural_log_exp_and_others table
    set (contains relu/copy/exp/ln) so the kernel pays exactly one
    ACT_TABLE_LOAD instead of thrashing between exp/ln sets. Entries keep
    their positions so act_func_set_id indices stay valid."""
    import concourse.bacc as _bacc

    if getattr(_bacc, "_ant_tables_patched", False):
        return
    orig = _bacc.get_activation_tables

    def patched(arch):
        tabs = orig(arch)
        if "natural_log_exp_and_others" not in tabs:
            return tabs
        return {
            k: (v if k == "natural_log_exp_and_others" else set())
            for k, v in tabs.items()
        }

    _bacc.get_activation_tables = patched
    _bacc._ant_tables_patched = True


def _build(bpc):
    _patch_act_tables()
    nc = bacc.Bacc(None, target_bir_lowering=False)

    # ---- DRAM I/O (per core) ----
    d = {}
    d["xfall"] = nc.dram_tensor("xfall", [bpc, 3, 2, 128, N], BF16, kind="ExternalInput")
    d["xnall"] = nc.dram_tensor("xnall", [bpc, 2, N, DIM], F32, kind="ExternalInput")
    # packed bf16 params: wp1,wp2,wp3,wkv1,wkv2,wkv3,we2 (1024 cols each) + idbf
    d["pbf"] = nc.dram_tensor("pbf", [128, 7 * 1024 + 128], BF16, kind="ExternalInput")
    # packed f32 params: fmb(6) + mask(128) + idf32(128)
    d["pf32"] = nc.dram_tensor("pf32", [128, 262], F32, kind="ExternalInput")
    d["we1"] = nc.dram_tensor("we1", [N, CLS], F32, kind="ExternalInput")
    # packed [1, x] rows: bf16 tbias(3*512) + be2row(512) + ones128(128)
    d["rbf"] = nc.dram_tensor("rbf", [3 * 512 + 512 + 128], BF16, kind="ExternalInput")
    # packed [1, x] f32r rows: be1(6) + ones256(256)
    d["rf32"] = nc.dram_tensor("rf32", [CLS + DIM], F32, kind="ExternalInput")
    # broadcast row: g1(6) b1(6) g2(256) b2(256)
    d["brow"] = nc.dram_tensor("brow", [2 * CLS + 2 * DIM], F32, kind="ExternalInput")
    d["out1"] = nc.dram_tensor("out1", [bpc, 2, 128, CLS], F32, kind="ExternalOutput")
    d["out2"] = nc.dram_tensor("out2", [bpc, N, DIM], F32, kind="ExternalOutput")

    with tile.TileContext(nc, pool_alloc_mode="queue") as tc, ExitStack() as ctx:
        _emit(nc, tc, ctx, d, bpc)
    nc.compile()
    return nc


def _emit(nc, tc, ctx, d, bpc):
    const = ctx.enter_context(tc.tile_pool(name="const", bufs=1))
    xin = ctx.enter_context(tc.tile_pool(name="xin", bufs=2))
    acts = ctx.enter_context(tc.tile_pool(name="acts", bufs=2))
    mid = ctx.enter_context(tc.tile_pool(name="mid", bufs=2))
    ypool = ctx.enter_context(tc.tile_pool(name="ypool", bufs=2))
    opool = ctx.enter_context(tc.tile_pool(name="opool", bufs=2))
    psm = ctx.enter_context(tc.tile_pool(name="psm", bufs=4, space="PSUM"))
    psb = ctx.enter_context(tc.tile_pool(name="psb", bufs=2, space="PSUM"))

    # ---- constants (loaded once, few big DMAs) ----
    pbf_t = const.tile([128, 7 * 1024 + 128], BF16, tag="pbf")
    nc.sync.dma_start(pbf_t[:], d["pbf"][:])
    wp_t = [pbf_t[:, i * 1024 : (i + 1) * 1024].rearrange("p (c f) -> p c f", c=2)
            for i in range(3)]
    wkv_t = [pbf_t[:, (3 + i) * 1024 : (4 + i) * 1024].rearrange("p (c f) -> p c f", c=2)
             for i in range(3)]
    we2_t = pbf_t[:, 6 * 1024 : 7 * 1024].rearrange("p (c f) -> p c f", c=4)
    idbf_t = pbf_t[:, 7 * 1024 : 7 * 1024 + 128]

    pf32_t = const.tile([128, 262], F32, tag="pf32")
    nc.sync.dma_start(pf32_t[:], d["pf32"][:])
    fmb_t = pf32_t[:, 0:6].rearrange("p (i t) -> p i t", i=3)
    mask_t = pf32_t[:, 6:134]
    idf32_t = pf32_t[:, 134:262]

    we1_t = const.tile([128, NT, CLS], F32R, tag="we1")
    nc.sync.dma_start(we1_t[:], d["we1"][:].rearrange("(t p) c -> p t c", p=128).bitcast(F32R))

    rbf_t = const.tile([1, 3 * 512 + 512 + 128], BF16, tag="rbf")
    nc.sync.dma_start(rbf_t[:], d["rbf"][:].unsqueeze(0))
    tbias_t = rbf_t[:, 0 : 3 * 512].rearrange("p (i f) -> p i f", i=3)
    be2row_t = rbf_t[:, 3 * 512 : 3 * 512 + 512]
    ones128_t = rbf_t[:, 4 * 512 : 4 * 512 + 128]

    rf32_t = const.tile([1, CLS + DIM], F32R, tag="rf32")
    nc.sync.dma_start(rf32_t[:], d["rf32"][:].unsqueeze(0).bitcast(F32R))
    be1_t = rf32_t[:, 0:CLS]
    ones256_t = rf32_t[:, CLS : CLS + DIM]

    brow_t = const.tile([128, 2 * CLS + 2 * DIM], F32, tag="brow")
    nc.gpsimd.dma_start(
        brow_t[:],
        bass.AP(tensor=d["brow"][:].tensor, offset=0, ap=[[0, 128], [1, 2 * CLS + 2 * DIM]]),
    )
    g1_t = brow_t[:, 0:CLS]
    b1_t = brow_t[:, CLS : 2 * CLS]
    g2_t = brow_t[:, 2 * CLS : 2 * CLS + DIM]
    b2_t = brow_t[:, 2 * CLS + DIM :]
    eps_t = const.tile([128, 1], F32, tag="eps")
    nc.vector.memset(eps_t[:], EPS)

    # ================= per-batch stage helpers =================

    def load_x(b):
        xfa = xin.tile([128, 3, 2, N], BF16, tag="xfa")
        nc.sync.dma_start(xfa[:], d["xfall"][b].rearrange("i c p n -> p i c n"))
        xna = xin.tile([128, 2, NT, DIM], F32, tag="xna")
        nc.sync.dma_start(xna[:], d["xnall"][b].rearrange("j (t p) c -> p j t c", p=128))
        return {"xf": xfa, "xn": xna}

    def proj(st, i):
        """Projection i: token-major half -> atok (+copy), fm half -> qfm."""
        xf = st["xf"]
        tok_lo = 0 if i < 2 else DIM
        at = acts.tile([128, NT, DIM], BF16, tag=f"atok{i}", bufs=1)
        for pair in range(NT // 2):
            ps = psm.tile([128, 512], F32, tag="m")
            for sub in range(2):
                nt = 2 * pair + sub
                dst = ps[:, sub * DIM : (sub + 1) * DIM]
                for ch in range(2):
                    nc.tensor.matmul(
                        dst,
                        xf[:, i, ch, nt * 128 : (nt + 1) * 128],
                        wp_t[i][:, ch, tok_lo : tok_lo + DIM],
                        start=(ch == 0),
                        stop=False,
                    )
                nc.tensor.matmul(
                    dst, ones128_t, tbias_t[:, i, 0:DIM], start=False, stop=True
                )
            dst_sb = at[:, 2 * pair : 2 * pair + 2, :].rearrange("p a b -> p (a b)")
            if pair % 2 == 0:
                nc.scalar.activation(dst_sb, ps[:], AF.Relu)
            else:
                nc.vector.tensor_scalar_max(dst_sb, ps[:], 0.0)
        # second copy so G's lhsT and rhs don't fight over one SBUF tile
        at2 = acts.tile([128, NT, DIM], BF16, tag=f"atok2_{i}", bufs=1)
        nc.vector.tensor_copy(
            at2[:].rearrange("p a b -> p (a b)"), at[:].rearrange("p a b -> p (a b)")
        )
        st[f"atok{i}"] = at
        st[f"atok2_{i}"] = at2

        fm_lo = DIM if i < 2 else 0
        qt = acts.tile([128, 2, N], BF16, tag=f"qfm{i}")
        for cc in range(2):
            ps = psb.tile([128, N], F32, tag="big")
            for nh in range(2):
                dst = ps[:, nh * 512 : (nh + 1) * 512]
                for ch in range(2):
                    nc.tensor.matmul(
                        dst,
                        wp_t[i][:, ch, fm_lo + cc * 128 : fm_lo + (cc + 1) * 128],
                        xf[:, i, ch, nh * 512 : (nh + 1) * 512],
                        start=(ch == 0),
                        stop=(ch == 1),
                    )
            nc.scalar.activation(
                qt[:, cc, 0:512], ps[:, 0:512], AF.Relu,
                bias=fmb_t[:, i, cc : cc + 1],
            )
            nc.vector.tensor_scalar(
                out=qt[:, cc, 512:N], in0=ps[:, 512:N],
                scalar1=fmb_t[:, i, cc : cc + 1], scalar2=0.0,
                op0=OP.add, op1=OP.max,
            )
        st[f"qfm{i}"] = qt

    def gram_ctx(st, i):
        """G -> T2 -> ctxT -> softmax (vector). PE stays dense."""
        at, at2 = st[f"atok{i}"], st[f"atok2_{i}"]
        gps = psm.tile([128, 512], F32, tag="m")
        for s in range(2):
            dst = gps[:, s * DIM : (s + 1) * DIM]
            for nt in range(NT):
                nc.tensor.matmul(
                    dst,
                    at2[:, nt, s * 128 : (s + 1) * 128],
                    at[:, nt, :],
                    start=(nt == 0),
                    stop=(nt == NT - 1),
                )
        gsb = mid.tile([128, 2, DIM], BF16, tag="gsb")
        nc.scalar.activation(gsb[:].rearrange("p a b -> p (a b)"), gps[:], AF.Copy)

        t2ps = psm.tile([128, 512], F32, tag="m")
        for s in range(2):
            dst = t2ps[:, s * DIM : (s + 1) * DIM]
            for ch in range(2):
                nc.tensor.matmul(
                    dst,
                    gsb[:, ch, s * 128 : (s + 1) * 128],
                    wkv_t[i][:, ch, 0:DIM],
                    start=(ch == 0),
                    stop=(ch == 1),
                )
        t2sb = mid.tile([128, 2, DIM], BF16, tag="t2sb")
        nc.vector.tensor_copy(t2sb[:].rearrange("p a b -> p (a b)"), t2ps[:])

        cxT = mid.tile([128, 2, 128], BF16, tag=f"cxT{i}")
        for h in range(2):
            cps = psm.tile([128, 128], F32, tag="m")
            for ch in range(2):
                nc.tensor.matmul(
                    cps[:],
                    wkv_t[i][:, ch, DIM + h * 128 : DIM + (h + 1) * 128],
                    t2sb[:, ch, h * 128 : (h + 1) * 128],
                    start=(ch == 0),
                    stop=(ch == 1),
                )
            # softmax over d (free dim), per 32-block via additive mask
            tmp = mid.tile([128, 128], F32, tag="smx")
            nc.vector.scalar_tensor_tensor(
                out=tmp[:], in0=cps[:], scalar=SCALE, in1=mask_t,
                op0=OP.mult, op1=OP.add,
            )
            pex = mid.tile([128, 128], F32, tag="pex")
            ssum = mid.tile([128, 1], F32, tag="ssum")
            nc.scalar.activation(pex[:], tmp[:], AF.Exp, accum_out=ssum[:])
            rs = mid.tile([128, 1], F32, tag="rs")
            nc.vector.reciprocal(rs[:], ssum[:])
            if i == 0:
                nc.vector.tensor_scalar(
                    out=cxT[:, h, :], in0=pex[:], scalar1=rs[:], scalar2=0.5,
                    op0=OP.mult, op1=OP.mult,
                )
            else:
                nc.vector.tensor_scalar_mul(cxT[:, h, :], pex[:], rs[:])
        st[f"cxT{i}"] = cxT

    def ctx_transpose(st, i):
        cxsb = mid.tile([128, 2, 128], BF16, tag=f"cxsb{i}")
        for h in range(2):
            tps = psm.tile([128, 128], BF16, tag="m")
            nc.tensor.transpose(tps[:], st[f"cxT{i}"][:, h, :], idbf_t)
            nc.vector.tensor_copy(cxsb[:, h, :], tps[:])
        st[f"ctx{i}"] = cxsb

    def fm_attend(st, qi, ci, base):
        y2o = st["y2o"]
        for h in range(2):
            ps = psb.tile([128, N], F32, tag="big")
            for nh in range(2):
                nc.tensor.matmul(
                    ps[:, nh * 512 : (nh + 1) * 512],
                    st[f"ctx{ci}"][:, h, :],
                    st[f"qfm{qi}"][:, h, nh * 512 : (nh + 1) * 512],
                    start=True, stop=True,
                )
            nc.scalar.activation(y2o[:, base + h, 0:512], ps[:, 0:512], AF.Copy)
            nc.vector.tensor_copy(y2o[:, base + h, 512:N], ps[:, 512:N])

    def tok_attend(st):
        """v1+z1 accumulated token-major; y1o = . + x1."""
        y1o = ypool.tile([128, NT, DIM], F32R, tag="y1o", bufs=1)
        xn = st["xn"]
        for pair in range(NT // 2):
            ps = psm.tile([128, 512], F32, tag="m")
            for sub in range(2):
                nt = 2 * pair + sub
                for h in range(2):
                    dst = ps[:, sub * DIM + h * 128 : sub * DIM + (h + 1) * 128]
                    nc.tensor.matmul(
                        dst, st["qfm0"][:, h, nt * 128 : (nt + 1) * 128],
                        st["ctx2"][:, h, :], start=True, stop=False,
                    )
                    nc.tensor.matmul(
                        dst, st["qfm2"][:, h, nt * 128 : (nt + 1) * 128],
                        st["ctx0"][:, h, :], start=False, stop=True,
                    )
            nc.vector.tensor_add(
                y1o[:, 2 * pair : 2 * pair + 2, :].rearrange("p a b -> p (a b)"),
                ps[:],
                xn[:, 0, 2 * pair : 2 * pair + 2, :].rearrange("p a b -> p (a b)"),
            )
        st["y1o"] = y1o

    def end1(st, b):
        """t^T = We1^T @ y1o + be1 -> transpose -> LN over CLS -> out1."""
        y1o = st["y1o"]
        tps = psm.tile([CLS, DIM], F32, tag="m")
        for nt in range(NT):
            nc.tensor.matmul(
                tps[:], we1_t[:, nt, :], y1o[:, nt, :], start=(nt == 0), stop=False
            )
        nc.tensor.matmul(tps[:], be1_t, ones256_t, start=False, stop=True)
        tsb = mid.tile([CLS, DIM], F32, tag="tsb")
        nc.scalar.activation(tsb[:], tps[:], AF.Copy)
        mv1 = mid.tile([128, 2, 2], F32, tag="mv1")
        trp = []
        for half in range(2):
            tp = psm.tile([128, CLS], F32, tag="m")
            nc.tensor.transpose(tp[:], tsb[:, half * 128 : (half + 1) * 128], idf32_t[:CLS, :CLS])
            trp.append(tp)
            st6 = mid.tile([128, 6], F32, tag="st6")
            nc.vector.bn_stats(st6[:], tp[:])
            nc.vector.bn_aggr(mv1[:, half, :], st6[:])
        lnv1 = mid.tile([128, 2], F32, tag="lnv1")
        nc.scalar.activation(lnv1[:], mv1[:, :, 1], AF.Ln, bias=eps_t[:])
        rstd1 = mid.tile([128, 2], F32, tag="rstd1")
        nc.scalar.activation(rstd1[:], lnv1[:], AF.Exp, scale=-0.5)
        for half in range(2):
            u = mid.tile([128, CLS], F32, tag="u6")
            nc.vector.scalar_tensor_tensor(
                out=u[:], in0=trp[half][:], scalar=mv1[:, half, 0:1], in1=g1_t,
                op0=OP.subtract, op1=OP.mult,
            )
            o1 = opool.tile([128, CLS], F32, tag="o1")
            nc.vector.scalar_tensor_tensor(
                out=o1[:], in0=u[:], scalar=rstd1[:, half : half + 1], in1=b1_t,
                op0=OP.mult, op1=OP.add,
            )
            nc.sync.dma_start(d["out1"][b, half], o1[:])

    def end2(st, b):
        """p2 = y2o @ We2 + be2 ; out2 = LN(x2 + p2), rstd per pair."""
        y2o, xn = st["y2o"], st["xn"]
        o2 = opool.tile([128, NT, DIM], F32, tag="o2")
        vsb = ypool.tile([128, NT, DIM], F32, tag="vsb", bufs=1)
        mv2 = mid.tile([128, NT, 2], F32, tag="mv2")
        for pair in range(NT // 2):
            ps = psm.tile([128, 512], F32, tag="m")
            for sub in range(2):
                nt = 2 * pair + sub
                dst = ps[:, sub * DIM : (sub + 1) * DIM]
                for ck in range(4):
                    nc.tensor.matmul(
                        dst,
                        y2o[:, ck, nt * 128 : (nt + 1) * 128],
                        we2_t[:, ck, :],
                        start=(ck == 0),
                        stop=False,
                    )
                nc.tensor.matmul(
                    dst, ones128_t, be2row_t[:, 0:DIM], start=False, stop=True
                )
            vs = vsb[:, 2 * pair : 2 * pair + 2, :].rearrange("p a b -> p (a b)")
            nc.vector.scalar_tensor_tensor(
                out=vs, in0=ps[:], scalar=0.0,
                in1=xn[:, 1, 2 * pair : 2 * pair + 2, :].rearrange("p a b -> p (a b)"),
                op0=OP.bypass, op1=OP.add,
            )
            for sub in range(2):
                nt = 2 * pair + sub
                stt = mid.tile([128, 6], F32, tag="st2")
                nc.vector.bn_stats(stt[:], vsb[:, nt, :])
                nc.vector.bn_aggr(mv2[:, nt, :], stt[:])
            lnv2 = mid.tile([128, 2], F32, tag="lnv2")
            nc.scalar.activation(
                lnv2[:], mv2[:, 2 * pair : 2 * pair + 2, 1], AF.Ln, bias=eps_t[:]
            )
            rstd2 = mid.tile([128, 2], F32, tag="rstd2")
            nc.scalar.activation(rstd2[:], lnv2[:], AF.Exp, scale=-0.5)
            for sub in range(2):
                nt = 2 * pair + sub
                u = mid.tile([128, DIM], F32, tag="u2")
                nc.vector.scalar_tensor_tensor(
                    out=u[:], in0=vsb[:, nt, :], scalar=mv2[:, nt, 0:1], in1=g2_t,
                    op0=OP.subtract, op1=OP.mult,
                )
                nc.vector.scalar_tensor_tensor(
                    out=o2[:, nt, :], in0=u[:], scalar=rstd2[:, sub : sub + 1], in1=b2_t,
                    op0=OP.mult, op1=OP.add,
                )
        nc.sync.dma_start(d["out2"][b].rearrange("(t p) c -> p t c", p=128), o2[:])

    def front(b):
        st = load_x(b)
        for i in range(3):
            proj(st, i)
        for i in range(3):
            gram_ctx(st, i)
        return st

    def back(st, b):
        st["y2o"] = ypool.tile([128, 4, N], BF16, tag="y2o", bufs=1, name="y2o")
        ctx_transpose(st, 0)
        ctx_transpose(st, 1)
        fm_attend(st, 2, 1, 0)   # z2 = y3 @ ctx2
        ctx_transpose(st, 2)
        fm_attend(st, 1, 2, 2)   # v2 = u2 @ ctx3
        tok_attend(st)
        end1(st, b)
        end2(st, b)

    # ---- software pipeline: back(b-1) interleaves into front(b) ----
    def front_interleaved(b, pst):
        st = load_x(b)
        proj(st, 0)
        if pst is not None:
            pst["y2o"] = ypool.tile([128, 4, N], BF16, tag="y2o", bufs=1, name="y2o")
            ctx_transpose(pst, 0)
            ctx_transpose(pst, 1)
            fm_attend(pst, 2, 1, 0)
            ctx_transpose(pst, 2)
            fm_attend(pst, 1, 2, 2)
        proj(st, 1)
        if pst is not None:
            tok_attend(pst)
        proj(st, 2)
        if pst is not None:
            end1(pst, pst["b"])
        gram_ctx(st, 0)
        if pst is not None:
            end2(pst, pst["b"])
        gram_ctx(st, 1)
        gram_ctx(st, 2)
        return st

    pst = None
    for b in range(bpc):
        st = front_interleaved(b, pst)
        st["b"] = b
        pst = st
    back(pst, pst["b"])


def _prep_params(inp):
    """Host-side param prep shared by all cores."""
    f = lambda a: np.ascontiguousarray(a, dtype=np.float32)
    bf = lambda a: np.ascontiguousarray(np.asarray(a, dtype=np.float32).astype(BF))
    wp1 = np.asarray(inp["Wp1"], np.float32).copy()
    wp1[:, DIM:] *= 0.5  # fold (z1+v1)/2's 0.5 into u1 = relu(x1 @ Wp1_u + bp1_u)

    # pbf: [128, 7*1024 + 128] bf16
    cols = []
    for w in (wp1, inp["Wp2"], inp["Wp3"], inp["Wkv1"], inp["Wkv2"], inp["Wkv3"]):
        # [256, 512] -> [128, 2, 512] -> [128, 1024] with [p, ch*512+f] = W[ch*128+p, f]
        cols.append(np.asarray(w, np.float32).reshape(2, 128, 512).transpose(1, 0, 2).reshape(128, 1024))
    we2 = np.asarray(inp["We2"], np.float32).reshape(4, 128, DIM).transpose(1, 0, 2).reshape(128, 1024)
    cols.append(we2)
    cols.append(np.eye(128, dtype=np.float32))
    p = {"pbf": bf(np.concatenate(cols, axis=1))}

    # pf32: fmb(6) + mask(128) + idf32(128)
    bp1, bp2, bp3 = (np.asarray(inp[k], np.float32) for k in ("bp1", "bp2", "bp3"))
    fmb = np.stack([0.5 * bp1[DIM:], bp2[DIM:], bp3[0:DIM]])  # [3, 256]
    fmb = fmb.reshape(3, 2, 128).transpose(2, 0, 1).reshape(128, 6)
    ii, jj = np.meshgrid(np.arange(128), np.arange(128), indexing="ij")
    mask = np.where(ii // 32 == jj // 32, 0.0, MASKV).astype(np.float32)
    p["pf32"] = f(np.concatenate([fmb, mask, np.eye(128, dtype=np.float32)], axis=1))

    p["we1"] = f(inp["We1"])
    p["rbf"] = bf(np.concatenate([
        np.tile(bp1[0:DIM], 2), np.tile(bp2[0:DIM], 2), np.tile(bp3[DIM:], 2),
        np.tile(np.asarray(inp["be2"], np.float32), 2), np.ones(128, np.float32)]))
    p["rf32"] = f(np.concatenate([np.asarray(inp["be1"], np.float32), np.ones(DIM, np.float32)]))
    p["brow"] = f(np.concatenate([inp["g1"], inp["b1"], inp["g2"], inp["b2"]]))
    return p


def _run(inputs, trace=False):
    if "nc" not in _CACHE:
        _CACHE["nc"] = _build(BPC)
    nc = _CACHE["nc"]
    params = _prep_params(inputs)
    x1 = np.asarray(inputs["x1"], np.float32)
    x2 = np.asarray(inputs["x2"], np.float32)
    sg = np.asarray(inputs["segfeature"], np.float32)
    # [B, 3, 2, 128, N] bf16 transposed inputs
    xf = np.stack([x1, x2, sg], axis=1).transpose(0, 1, 3, 2).reshape(B, 3, 2, 128, N).astype(BF)
    xn = np.stack([x1, x2], axis=1)  # [B, 2, N, DIM]

    in_maps = []
    for c in range(N_CORES):
        lo, hi = c * BPC, (c + 1) * BPC
        m = dict(params)
        m["xfall"] = np.ascontiguousarray(xf[lo:hi])
        m["xnall"] = np.ascontiguousarray(xn[lo:hi])
        in_maps.append(m)

    res = run_bass_kernel_spmd(nc, in_maps, core_ids=list(range(N_CORES)), trace=trace)
    out1 = np.concatenate([r["out1"].reshape(BPC, 2 * 128, CLS) for r in res.results])
    out2 = np.concatenate([r["out2"] for r in res.results])
    out_x1 = np.ascontiguousarray(np.swapaxes(out1, 1, 2), dtype=np.float32)
    return (out_x1, out2.astype(np.float32, copy=False)), res


def kernel(**inputs):
    outs, _ = _run(inputs, trace=False)
    return outs


# revision 19
# speedup vs baseline: 1.3632x; 1.3632x over previous
"""Trainium2 Bass kernel for nn_CrossPath (sparse_attention).

Strategy (hardcoded for B=32, N=1024, C=256, H=8, d=32, CLS=6):
  - Data-parallel over batch: 8 NeuronCores x 4 batches each, params replicated.
  - All heavy matmuls run on the PE at 1 cycle/row: bf16 for proj/attention
    paths, float32r (tf32-like) for the precision-sensitive end_proj1.
  - The linear-attention context is computed via the Gram-matrix identity
        ctx = Wk^T (act^T act) Wv * scale
    which cuts both PE work and PSUM-evacuation volume vs materializing k/v.
  - Per-head softmax over d is done on [128,128] feature blocks with an
    additive block-diagonal mask; off-diagonal (cross-head) entries go to
    exp(-100)=0, so the transposed context is exactly block-diagonal and
    attends can use full 128-wide contractions.
  - Batches are software-pipelined: batch b's attention/output tail is
    interleaved into batch b+1's projection phase so the in-order PE never
    waits on vector-engine work.
  - Host-side glue: shard/transpose/cast/pack inputs, fold constant scales
    ((z1+v1)/2 -> 0.5 into ctx1 norm and Wp1's u-half), unshard outputs.
"""

import sys

sys.path.insert(0, "/opt/trn_rl_repo")

import numpy as np
import ml_dtypes
from contextlib import ExitStack

import concourse.bass as bass
import concourse.bacc as bacc
import concourse.tile as tile
import concourse.mybir as mybir
from concourse.bass_utils import run_bass_kernel_spmd

F32 = mybir.dt.float32
F32R = mybir.dt.float32r
BF16 = mybir.dt.bfloat16
AF = mybir.ActivationFunctionType
OP = mybir.AluOpType
BF = ml_dtypes.bfloat16

N_CORES = 8
B = 32
BPC = B // N_CORES  # batches per core
DIM = 256
N = 1024
H = 8
CLS = 6
SCALE = float((DIM // H) ** -0.5)
EPS = 1e-5
NT = N // 128  # 8 token tiles
MASKV = -100.0

_CACHE = {}


def _patch_act_tables():
    """Force all ACT functions onto the natural_log_exp_and_others table
    set (contains relu/copy/exp/ln) so the kernel pays exactly one
    ACT_TABLE_LOAD instead of thrashing between exp/ln sets. Entries keep
    their positions so act_func_set_id indices stay valid."""
    import concourse.bacc as _bacc

    if getattr(_bacc, "_ant_tables_patched", False):
        return
    orig = _bacc.get_activation_tables

    def patched(arch):
        tabs = orig(arch)
        if "natural_log_exp_and_others" not in tabs:
            return tabs
        return {
            k: (v if k == "natural_log_exp_and_others" else set())
            for k, v in tabs.items()
        }

    _bacc.get_activation_tables = patched
    _bacc._ant_tables_patched = True


def _build(bpc):
    _patch_act_tables()
    nc = bacc.Bacc(None, target_bir_lowering=False)

    # ---- DRAM I/O (per core) ----
    d = {}
    d["xfall"] = nc.dram_tensor("xfall", [bpc, 3, 2, 128, N], BF16, kind="ExternalInput")
    d["xnall"] = nc.dram_tensor("xnall", [bpc, 2, N, DIM], F32, kind="ExternalInput")
    # packed bf16 params: wp1,wp2,wp3,wkv1,wkv2,wkv3,we2 (1024 cols each) + idbf
    d["pbf"] = nc.dram_tensor("pbf", [128, 7 * 1024 + 128], BF16, kind="ExternalInput")
    # packed f32 params: fmb(6) + mask(128) + idf32(128)
    d["pf32"] = nc.dram_tensor("pf32", [128, 262], F32, kind="ExternalInput")
    d["we1"] = nc.dram_tensor("we1", [N, CLS], F32, kind="ExternalInput")
    # packed [1, x] rows: bf16 tbias(3*512) + be2row(512) + ones128(128)
    d["rbf"] = nc.dram_tensor("rbf", [3 * 512 + 512 + 128], BF16, kind="ExternalInput")
    # packed [1, x] f32r rows: be1(6) + ones256(256)
    d["rf32"] = nc.dram_tensor("rf32", [CLS + DIM], F32, kind="ExternalInput")
    # broadcast row: g1(6) b1(6) g2(256) b2(256)
    d["brow"] = nc.dram_tensor("brow", [2 * CLS + 2 * DIM], F32, kind="ExternalInput")
    d["out1"] = nc.dram_tensor("out1", [bpc, 2, 128, CLS], F32, kind="ExternalOutput")
    d["out2"] = nc.dram_tensor("out2", [bpc, N, DIM], F32, kind="ExternalOutput")

    with tile.TileContext(nc, pool_alloc_mode="queue") as tc, ExitStack() as ctx:
        _emit(nc, tc, ctx, d, bpc)
    nc.compile()
    return nc


def _emit(nc, tc, ctx, d, bpc):
    const = ctx.enter_context(tc.tile_pool(name="const", bufs=1))
    xin = ctx.enter_context(tc.tile_pool(name="xin", bufs=2))
    acts = ctx.enter_context(tc.tile_pool(name="acts", bufs=2))
    mid = ctx.enter_context(tc.tile_pool(name="mid", bufs=2))
    ypool = ctx.enter_context(tc.tile_pool(name="ypool", bufs=2))
    opool = ctx.enter_context(tc.tile_pool(name="opool", bufs=2))
    psm = ctx.enter_context(tc.tile_pool(name="psm", bufs=6, space="PSUM"))
    ps2 = ctx.enter_context(tc.tile_pool(name="ps2", bufs=2, space="PSUM"))

    # ---- constants (loaded once, few big DMAs) ----
    pbf_t = const.tile([128, 7 * 1024 + 128], BF16, tag="pbf")
    nc.sync.dma_start(pbf_t[:], d["pbf"][:])
    wp_t = [pbf_t[:, i * 1024 : (i + 1) * 1024].rearrange("p (c f) -> p c f", c=2)
            for i in range(3)]
    wkv_t = [pbf_t[:, (3 + i) * 1024 : (4 + i) * 1024].rearrange("p (c f) -> p c f", c=2)
             for i in range(3)]
    we2_t = pbf_t[:, 6 * 1024 : 7 * 1024].rearrange("p (c f) -> p c f", c=4)
    idbf_t = pbf_t[:, 7 * 1024 : 7 * 1024 + 128]

    pf32_t = const.tile([128, 262], F32, tag="pf32")
    nc.sync.dma_start(pf32_t[:], d["pf32"][:])
    fmb_t = pf32_t[:, 0:6].rearrange("p (i t) -> p i t", i=3)
    mask_t = pf32_t[:, 6:134]
    idf32_t = pf32_t[:, 134:262]

    we1_t = const.tile([128, NT, CLS], F32R, tag="we1")
    nc.sync.dma_start(we1_t[:], d["we1"][:].rearrange("(t p) c -> p t c", p=128).bitcast(F32R))

    rbf_t = const.tile([1, 3 * 512 + 512 + 128], BF16, tag="rbf")
    nc.sync.dma_start(rbf_t[:], d["rbf"][:].unsqueeze(0))
    tbias_t = rbf_t[:, 0 : 3 * 512].rearrange("p (i f) -> p i f", i=3)
    be2row_t = rbf_t[:, 3 * 512 : 3 * 512 + 512]
    ones128_t = rbf_t[:, 4 * 512 : 4 * 512 + 128]

    rf32_t = const.tile([1, CLS + DIM], F32R, tag="rf32")
    nc.sync.dma_start(rf32_t[:], d["rf32"][:].unsqueeze(0).bitcast(F32R))
    be1_t = rf32_t[:, 0:CLS]
    ones256_t = rf32_t[:, CLS : CLS + DIM]

    brow_t = const.tile([128, 2 * CLS + 2 * DIM], F32, tag="brow")
    nc.gpsimd.dma_start(
        brow_t[:],
        bass.AP(tensor=d["brow"][:].tensor, offset=0, ap=[[0, 128], [1, 2 * CLS + 2 * DIM]]),
    )
    g1_t = brow_t[:, 0:CLS]
    b1_t = brow_t[:, CLS : 2 * CLS]
    g2_t = brow_t[:, 2 * CLS : 2 * CLS + DIM]
    b2_t = brow_t[:, 2 * CLS + DIM :]
    eps_t = const.tile([128, 1], F32, tag="eps")
    nc.vector.memset(eps_t[:], EPS)

    # ================= per-batch stage helpers =================

    def load_x(b):
        xfa = xin.tile([128, 3, 2, N], BF16, tag="xfa")
        nc.sync.dma_start(xfa[:], d["xfall"][b].rearrange("i c p n -> p i c n"))
        xna = xin.tile([128, 2, NT, DIM], F32, tag="xna")
        nc.sync.dma_start(xna[:], d["xnall"][b].rearrange("j (t p) c -> p j t c", p=128))
        return {"xf": xfa, "xn": xna}

    def proj(st, i):
        """Projection i: token-major half -> atok (+copy), fm half -> qfm."""
        xf = st["xf"]
        tok_lo = 0 if i < 2 else DIM
        at = acts.tile([128, NT, DIM], BF16, tag=f"atok{i}", bufs=1)
        for pair in range(NT // 2):
            ps = psm.tile([128, 512], F32, tag="m")
            for sub in range(2):
                nt = 2 * pair + sub
                dst = ps[:, sub * DIM : (sub + 1) * DIM]
                for ch in range(2):
                    nc.tensor.matmul(
                        dst,
                        xf[:, i, ch, nt * 128 : (nt + 1) * 128],
                        wp_t[i][:, ch, tok_lo : tok_lo + DIM],
                        start=(ch == 0),
                        stop=False,
                    )
                nc.tensor.matmul(
                    dst, ones128_t, tbias_t[:, i, 0:DIM], start=False, stop=True
                )
            dst_sb = at[:, 2 * pair : 2 * pair + 2, :].rearrange("p a b -> p (a b)")
            nc.scalar.activation(dst_sb, ps[:], AF.Relu)
        # second copy so G's lhsT and rhs don't fight over one SBUF tile
        at2 = acts.tile([128, NT, DIM], BF16, tag=f"atok2_{i}", bufs=1)
        nc.vector.tensor_copy(
            at2[:].rearrange("p a b -> p (a b)"), at[:].rearrange("p a b -> p (a b)")
        )
        st[f"atok{i}"] = at
        st[f"atok2_{i}"] = at2

        fm_lo = DIM if i < 2 else 0
        qt = acts.tile([128, 2, N], BF16, tag=f"qfm{i}")
        for cc in range(2):
            for nh in range(2):
                ps = psm.tile([128, 512], F32, tag="m")
                for ch in range(2):
                    nc.tensor.matmul(
                        ps[:],
                        wp_t[i][:, ch, fm_lo + cc * 128 : fm_lo + (cc + 1) * 128],
                        xf[:, i, ch, nh * 512 : (nh + 1) * 512],
                        start=(ch == 0),
                        stop=(ch == 1),
                    )
                dst = qt[:, cc, nh * 512 : (nh + 1) * 512]
                if nh == 0:
                    nc.scalar.activation(
                        dst, ps[:], AF.Relu, bias=fmb_t[:, i, cc : cc + 1]
                    )
                else:
                    nc.vector.tensor_scalar(
                        out=dst, in0=ps[:],
                        scalar1=fmb_t[:, i, cc : cc + 1], scalar2=0.0,
                        op0=OP.add, op1=OP.max,
                    )
        st[f"qfm{i}"] = qt

    def gram_ctx(st, i):
        """G -> T2 -> ctxT -> softmax (vector). PE stays dense."""
        at, at2 = st[f"atok{i}"], st[f"atok2_{i}"]
        gps = psm.tile([128, 512], F32, tag="m")
        for s in range(2):
            dst = gps[:, s * DIM : (s + 1) * DIM]
            for nt in range(NT):
                nc.tensor.matmul(
                    dst,
                    at2[:, nt, s * 128 : (s + 1) * 128],
                    at[:, nt, :],
                    start=(nt == 0),
                    stop=(nt == NT - 1),
                )
        gsb = mid.tile([128, 2, DIM], BF16, tag="gsb")
        nc.scalar.activation(gsb[:].rearrange("p a b -> p (a b)"), gps[:], AF.Copy)

        t2ps = psm.tile([128, 512], F32, tag="m")
        for s in range(2):
            dst = t2ps[:, s * DIM : (s + 1) * DIM]
            for ch in range(2):
                nc.tensor.matmul(
                    dst,
                    gsb[:, ch, s * 128 : (s + 1) * 128],
                    wkv_t[i][:, ch, 0:DIM],
                    start=(ch == 0),
                    stop=(ch == 1),
                )
        t2sb = mid.tile([128, 2, DIM], BF16, tag="t2sb")
        nc.scalar.activation(t2sb[:].rearrange("p a b -> p (a b)"), t2ps[:], AF.Copy)

        cxT = mid.tile([128, 2, 128], BF16, tag=f"cxT{i}")
        for h in range(2):
            cps = psm.tile([128, 128], F32, tag="m")
            for ch in range(2):
                nc.tensor.matmul(
                    cps[:],
                    wkv_t[i][:, ch, DIM + h * 128 : DIM + (h + 1) * 128],
                    t2sb[:, ch, h * 128 : (h + 1) * 128],
                    start=(ch == 0),
                    stop=(ch == 1),
                )
            # softmax over d (free dim), per 32-block via additive mask
            tmp = mid.tile([128, 128], F32, tag="smx")
            nc.vector.scalar_tensor_tensor(
                out=tmp[:], in0=cps[:], scalar=SCALE, in1=mask_t,
                op0=OP.mult, op1=OP.add,
            )
            pex = mid.tile([128, 128], F32, tag="pex")
            ssum = mid.tile([128, 1], F32, tag="ssum")
            nc.scalar.activation(pex[:], tmp[:], AF.Exp, accum_out=ssum[:])
            rs = mid.tile([128, 1], F32, tag="rs")
            nc.vector.reciprocal(rs[:], ssum[:])
            if i == 0:
                nc.vector.tensor_scalar(
                    out=cxT[:, h, :], in0=pex[:], scalar1=rs[:], scalar2=0.5,
                    op0=OP.mult, op1=OP.mult,
                )
            else:
                nc.vector.tensor_scalar_mul(cxT[:, h, :], pex[:], rs[:])
        st[f"cxT{i}"] = cxT

    def ctx_transpose(st, i):
        cxsb = mid.tile([128, 2, 128], BF16, tag=f"cxsb{i}")
        for h in range(2):
            tps = psm.tile([128, 128], BF16, tag="m")
            nc.tensor.transpose(tps[:], st[f"cxT{i}"][:, h, :], idbf_t)
            nc.vector.tensor_copy(cxsb[:, h, :], tps[:])
        st[f"ctx{i}"] = cxsb

    def fm_attend(st, qi, ci, base):
        y2o = st["y2o"]
        for h in range(2):
            for nh in range(2):
                ps = psm.tile([128, 512], F32, tag="m")
                nc.tensor.matmul(
                    ps[:],
                    st[f"ctx{ci}"][:, h, :],
                    st[f"qfm{qi}"][:, h, nh * 512 : (nh + 1) * 512],
                    start=True, stop=True,
                )
                dst = y2o[:, base + h, nh * 512 : (nh + 1) * 512]
                if nh == 0:
                    nc.scalar.activation(dst, ps[:], AF.Copy)
                else:
                    nc.vector.tensor_copy(dst, ps[:])

    def tok_attend(st):
        """v1+z1 accumulated token-major; y1o = . + x1."""
        y1o = ypool.tile([128, NT, DIM], F32R, tag="y1o", bufs=1)
        xn = st["xn"]
        for pair in range(NT // 2):
            ps = psm.tile([128, 512], F32, tag="m")
            for sub in range(2):
                nt = 2 * pair + sub
                for h in range(2):
                    dst = ps[:, sub * DIM + h * 128 : sub * DIM + (h + 1) * 128]
                    nc.tensor.matmul(
                        dst, st["qfm0"][:, h, nt * 128 : (nt + 1) * 128],
                        st["ctx2"][:, h, :], start=True, stop=False,
                    )
                    nc.tensor.matmul(
                        dst, st["qfm2"][:, h, nt * 128 : (nt + 1) * 128],
                        st["ctx0"][:, h, :], start=False, stop=True,
                    )
            nc.vector.tensor_add(
                y1o[:, 2 * pair : 2 * pair + 2, :].rearrange("p a b -> p (a b)"),
                ps[:],
                xn[:, 0, 2 * pair : 2 * pair + 2, :].rearrange("p a b -> p (a b)"),
            )
        st["y1o"] = y1o

    def end1(st, b):
        """t^T = We1^T @ y1o + be1 -> transpose -> LN over CLS -> out1."""
        y1o = st["y1o"]
        tps = psm.tile([CLS, DIM], F32, tag="m")
        for nt in range(NT):
            nc.tensor.matmul(
                tps[:], we1_t[:, nt, :], y1o[:, nt, :], start=(nt == 0), stop=False
            )
        nc.tensor.matmul(tps[:], be1_t, ones256_t, start=False, stop=True)
        tsb = mid.tile([CLS, DIM], F32, tag="tsb")
        nc.scalar.activation(tsb[:], tps[:], AF.Copy)
        mv1 = mid.tile([128, 2, 2], F32, tag="mv1")
        trp = []
        for half in range(2):
            tp = psm.tile([128, CLS], F32, tag="m")
            nc.tensor.transpose(tp[:], tsb[:, half * 128 : (half + 1) * 128], idf32_t[:CLS, :CLS])
            trp.append(tp)
            st6 = mid.tile([128, 6], F32, tag="st6")
            nc.vector.bn_stats(st6[:], tp[:])
            nc.vector.bn_aggr(mv1[:, half, :], st6[:])
        lnv1 = mid.tile([128, 2], F32, tag="lnv1")
        nc.scalar.activation(lnv1[:], mv1[:, :, 1], AF.Ln, bias=eps_t[:])
        rstd1 = mid.tile([128, 2], F32, tag="rstd1")
        nc.scalar.activation(rstd1[:], lnv1[:], AF.Exp, scale=-0.5)
        for half in range(2):
            u = mid.tile([128, CLS], F32, tag="u6")
            nc.vector.scalar_tensor_tensor(
                out=u[:], in0=trp[half][:], scalar=mv1[:, half, 0:1], in1=g1_t,
                op0=OP.subtract, op1=OP.mult,
            )
            o1 = opool.tile([128, CLS], F32, tag="o1")
            nc.vector.scalar_tensor_tensor(
                out=o1[:], in0=u[:], scalar=rstd1[:, half : half + 1], in1=b1_t,
                op0=OP.mult, op1=OP.add,
            )
            nc.sync.dma_start(d["out1"][b, half], o1[:])

    def end2(st, b):
        """p2 = y2o @ We2 + be2 ; out2 = LN(x2 + p2), rstd per pair."""
        y2o, xn = st["y2o"], st["xn"]
        o2 = opool.tile([128, NT, DIM], F32, tag="o2")
        vsb = ypool.tile([128, NT, DIM], F32, tag="vsb", bufs=1)
        mv2 = mid.tile([128, NT, 2], F32, tag="mv2")
        for pair in range(NT // 2):
            ps = ps2.tile([128, 512], F32, tag="p")
            for sub in range(2):
                nt = 2 * pair + sub
                dst = ps[:, sub * DIM : (sub + 1) * DIM]
                for ck in range(4):
                    nc.tensor.matmul(
                        dst,
                        y2o[:, ck, nt * 128 : (nt + 1) * 128],
                        we2_t[:, ck, :],
                        start=(ck == 0),
                        stop=False,
                    )
                nc.tensor.matmul(
                    dst, ones128_t, be2row_t[:, 0:DIM], start=False, stop=True
                )
            vs = vsb[:, 2 * pair : 2 * pair + 2, :].rearrange("p a b -> p (a b)")
            nc.vector.scalar_tensor_tensor(
                out=vs, in0=ps[:], scalar=0.0,
                in1=xn[:, 1, 2 * pair : 2 * pair + 2, :].rearrange("p a b -> p (a b)"),
                op0=OP.bypass, op1=OP.add,
            )
            for sub in range(2):
                nt = 2 * pair + sub
                stt = mid.tile([128, 6], F32, tag="st2")
                nc.vector.bn_stats(stt[:], vsb[:, nt, :])
                nc.vector.bn_aggr(mv2[:, nt, :], stt[:])
            lnv2 = mid.tile([128, 2], F32, tag="lnv2")
            nc.scalar.activation(
                lnv2[:], mv2[:, 2 * pair : 2 * pair + 2, 1], AF.Ln, bias=eps_t[:]
            )
            rstd2 = mid.tile([128, 2], F32, tag="rstd2")
            nc.scalar.activation(rstd2[:], lnv2[:], AF.Exp, scale=-0.5)
            for sub in range(2):
                nt = 2 * pair + sub
                u = mid.tile([128, DIM], F32, tag="u2")
                nc.vector.scalar_tensor_tensor(
                    out=u[:], in0=vsb[:, nt, :], scalar=mv2[:, nt, 0:1], in1=g2_t,
                    op0=OP.subtract, op1=OP.mult,
                )
                nc.vector.scalar_tensor_tensor(
                    out=o2[:, nt, :], in0=u[:], scalar=rstd2[:, sub : sub + 1], in1=b2_t,
                    op0=OP.mult, op1=OP.add,
                )
        nc.sync.dma_start(d["out2"][b].rearrange("(t p) c -> p t c", p=128), o2[:])

    def front(b):
        st = load_x(b)
        for i in range(3):
            proj(st, i)
        for i in range(3):
            gram_ctx(st, i)
        return st

    def back(st, b):
        st["y2o"] = ypool.tile([128, 4, N], BF16, tag="y2o", bufs=1, name="y2o")
        ctx_transpose(st, 0)
        ctx_transpose(st, 1)
        fm_attend(st, 2, 1, 0)   # z2 = y3 @ ctx2
        ctx_transpose(st, 2)
        fm_attend(st, 1, 2, 2)   # v2 = u2 @ ctx3
        tok_attend(st)
        end1(st, b)
        end2(st, b)

    # ---- software pipeline: back(b-1) interleaves into front(b) ----
    def front_interleaved(b, pst):
        st = load_x(b)
        proj(st, 0)
        if pst is not None:
            pst["y2o"] = ypool.tile([128, 4, N], BF16, tag="y2o", bufs=1, name="y2o")
            ctx_transpose(pst, 0)
            ctx_transpose(pst, 1)
            fm_attend(pst, 2, 1, 0)
            ctx_transpose(pst, 2)
            fm_attend(pst, 1, 2, 2)
        proj(st, 1)
        if pst is not None:
            tok_attend(pst)
        proj(st, 2)
        if pst is not None:
            end1(pst, pst["b"])
        gram_ctx(st, 0)
        if pst is not None:
            end2(pst, pst["b"])
        gram_ctx(st, 1)
        gram_ctx(st, 2)
        return st

    pst = None
    for b in range(bpc):
        st = front_interleaved(b, pst)
        st["b"] = b
        pst = st
    back(pst, pst["b"])


def _prep_params(inp):
    """Host-side param prep shared by all cores."""
    f = lambda a: np.ascontiguousarray(a, dtype=np.float32)
    bf = lambda a: np.ascontiguousarray(np.asarray(a, dtype=np.float32).astype(BF))
    wp1 = np.asarray(inp["Wp1"], np.float32).copy()
    wp1[:, DIM:] *= 0.5  # fold (z1+v1)/2's 0.5 into u1 = relu(x1 @ Wp1_u + bp1_u)

    # pbf: [128, 7*1024 + 128] bf16
    cols = []
    for w in (wp1, inp["Wp2"], inp["Wp3"], inp["Wkv1"], inp["Wkv2"], inp["Wkv3"]):
        # [256, 512] -> [128, 2, 512] -> [128, 1024] with [p, ch*512+f] = W[ch*128+p, f]
        cols.append(np.asarray(w, np.float32).reshape(2, 128, 512).transpose(1, 0, 2).reshape(128, 1024))
    we2 = np.asarray(inp["We2"], np.float32).reshape(4, 128, DIM).transpose(1, 0, 2).reshape(128, 1024)
    cols.append(we2)
    cols.append(np.eye(128, dtype=np.float32))
    p = {"pbf": bf(np.concatenate(cols, axis=1))}

    # pf32: fmb(6) + mask(128) + idf32(128)
    bp1, bp2, bp3 = (np.asarray(inp[k], np.float32) for k in ("bp1", "bp2", "bp3"))
    fmb = np.stack([0.5 * bp1[DIM:], bp2[DIM:], bp3[0:DIM]])  # [3, 256]
    fmb = fmb.reshape(3, 2, 128).transpose(2, 0, 1).reshape(128, 6)
    ii, jj = np.meshgrid(np.arange(128), np.arange(128), indexing="ij")
    mask = np.where(ii // 32 == jj // 32, 0.0, MASKV).astype(np.float32)
    p["pf32"] = f(np.concatenate([fmb, mask, np.eye(128, dtype=np.float32)], axis=1))

    p["we1"] = f(inp["We1"])
    p["rbf"] = bf(np.concatenate([
        np.tile(bp1[0:DIM], 2), np.tile(bp2[0:DIM], 2), np.tile(bp3[DIM:], 2),
        np.tile(np.asarray(inp["be2"], np.float32), 2), np.ones(128, np.float32)]))
    p["rf32"] = f(np.concatenate([np.asarray(inp["be1"], np.float32), np.ones(DIM, np.float32)]))
    p["brow"] = f(np.concatenate([inp["g1"], inp["b1"], inp["g2"], inp["b2"]]))
    return p


def _run(inputs, trace=False):
    if "nc" not in _CACHE:
        _CACHE["nc"] = _build(BPC)
    nc = _CACHE["nc"]
    params = _prep_params(inputs)
    x1 = np.asarray(inputs["x1"], np.float32)
    x2 = np.asarray(inputs["x2"], np.float32)
    sg = np.asarray(inputs["segfeature"], np.float32)
    # [B, 3, 2, 128, N] bf16 transposed inputs
    xf = np.stack([x1, x2, sg], axis=1).transpose(0, 1, 3, 2).reshape(B, 3, 2, 128, N).astype(BF)
    xn = np.stack([x1, x2], axis=1)  # [B, 2, N, DIM]

    in_maps = []
    for c in range(N_CORES):
        lo, hi = c * BPC, (c + 1) * BPC
        m = dict(params)
        m["xfall"] = np.ascontiguousarray(xf[lo:hi])
        m["xnall"] = np.ascontiguousarray(xn[lo:hi])
        in_maps.append(m)

    res = run_bass_kernel_spmd(nc, in_maps, core_ids=list(range(N_CORES)), trace=trace)
    out1 = np.concatenate([r["out1"].reshape(BPC, 2 * 128, CLS) for r in res.results])
    out2 = np.concatenate([r["out2"] for r in res.results])
    out_x1 = np.ascontiguousarray(np.swapaxes(out1, 1, 2), dtype=np.float32)
    return (out_x1, out2.astype(np.float32, copy=False)), res


def kernel(**inputs):
    outs, _ = _run(inputs, trace=False)
    return outs
